# revision 1
# baseline (speedup 1.0000x reference)
"""Causal self-attention on 8 TRN2 NeuronCores.

Problem: x[4, 2048, 1024], w_qkv[3072, 1024], w_proj[1024, 1024],
16 heads x 64 dims, causal softmax attention, output [4, 2048, 1024].

Sharding: core c handles (batch b = c//2, head-group hg = c%2).
Each head-group = 8 heads = 512 channels. Tensor-parallel over heads:
each core computes a *partial* projection output [2048, 1024]; the host
sums the two head-group partials per batch (the "all-reduce" of TP).

Per-core dataflow (all matmuls fp32r = full-rate PE):
  Phase A:  QT = Wq @ X^T   [512, 2048]   (head dims on partitions)
            KT = Wk @ X^T   [512, 2048]
            V  = X @ Wv^T   [2048, 512]   (+ ones column per head)
  Phase B (per 512-query block QI, per head h):
            ST_j = K_h^T Q_h  -> PSUM [128 keys, 512 queries]
            diagonal tiles:  ST += (-1e5 * I) @ staircase   (causal mask)
            PT_j = exp(0.125 * ST_j)            (ACT, masked lanes -> 0)
            YT  += [V_h | 1]^T @ PT_j           (accumulate over key tiles)
            row 64 of YT = softmax denominators (free-dim indexed)
  Normalize: r = 1/denoms; R = E_pc^T @ r broadcasts r over the 64
            partition rows of each head; YT *= R.
  Proj:     out = YT^T-contracted with w_proj slice -> [2048, 1024] partial.
"""

import numpy as np
from contextlib import ExitStack

import concourse.bass as bass
import concourse.tile as tile
from concourse import bacc, mybir
from concourse.bass_utils import run_bass_kernel_spmd

B, T, C, H, D = 4, 2048, 1024, 16, 64
HG = 2                 # head groups (tensor-parallel ways)
HPG = H // HG          # 8 heads per group
CG = HPG * D           # 512 channels per group
P = 128
NQI = T // 512         # 4 query blocks
NJT = T // P           # 16 key tiles
NEG = -1.0e5           # causal mask additive constant (exp(0.125*NEG) == 0)
F32 = mybir.dt.float32
F32R = mybir.dt.float32r

_CACHE = {}


def _build_core_program():
    nc = bacc.Bacc("TRN2", target_bir_lowering=False, debug=False, num_devices=8)
    xt = nc.dram_tensor("xt", [C, T], F32R, kind="ExternalInput").ap()
    wqkvt = nc.dram_tensor("wqkvt", [C, 3 * CG], F32R, kind="ExternalInput").ap()
    wpt = nc.dram_tensor("wpt", [CG, C], F32R, kind="ExternalInput").ap()
    out = nc.dram_tensor("out", [T, C], F32, kind="ExternalOutput").ap()

    with tile.TileContext(nc) as tc:
        with ExitStack() as ctx:
            _attention(ctx, tc, xt, wqkvt, wpt, out)
    nc.compile()
    return nc


def _attention(ctx, tc, xt, wqkvt, wpt, out):
    nc = tc.nc

    persist = ctx.enter_context(tc.tile_pool(name="persist", bufs=1))
    qt = persist.tile([P, 4, T], F32R, tag="qt")       # QT[c*128+p, i] at [p, c, i]
    kt = persist.tile([P, 4, T], F32R, tag="kt")
    v = persist.tile([P, NJT, HPG * 65], F32R, tag="v")  # [V_h | 1] per key tile
    ytu = persist.tile([P, 4, T], F32R, tag="ytu")     # normalized YT

    consts = ctx.enter_context(tc.tile_pool(name="consts", bufs=1))
    cstage_ctx = ExitStack()
    stage = cstage_ctx.enter_context(tc.tile_pool(name="cstage", bufs=2))
    # staircase "not-valid" masks for the 4 diagonal offsets.
    # affine_select can't write f32r directly (verifier wants a rounding
    # producer), so build each const in f32 and DVE-copy into f32r.
    notvalid = []
    for o in range(4):
        ms = stage.tile([P, 512], F32, tag="cst", name=f"nvs{o}")
        nc.gpsimd.memset(ms, 0.0)
        # keep 0 where (y - p - 128*o) >= 0 (valid), else fill 1 (masked)
        nc.gpsimd.affine_select(
            out=ms, in_=ms, compare_op=mybir.AluOpType.is_ge, fill=1.0,
            base=-128 * o, pattern=[[1, 512]], channel_multiplier=-1,
        )
        m = consts.tile([P, 512], F32R, tag=f"nv{o}", name=f"nv{o}")
        nc.vector.tensor_copy(m, ms)
        notvalid.append(m)
    ns = stage.tile([P, P], F32, tag="cst", name="negIs")
    nc.gpsimd.memset(ns, 0.0)
    nc.gpsimd.affine_select(
        out=ns, in_=ns, compare_op=mybir.AluOpType.not_equal, fill=NEG,
        base=0, pattern=[[-1, P]], channel_multiplier=1,
    )
    neg_i = consts.tile([P, P], F32R, tag="negI")
    nc.vector.tensor_copy(neg_i, ns)
    # E matrices: e8[pc][h, c] = 1 iff chunk-pc channel c belongs to head h
    e8 = []
    for pc in range(4):
        es = stage.tile([8, P], F32, tag="cste", name=f"e8s{pc}")
        nc.gpsimd.memset(es, 0.0)
        # row h, cols: head index of col c is 2*pc + c//64 -> fill 1 on match:
        # (h - 2*pc - c//64) == 0.  pattern [[-1,2],[0,64]] over free [2, 64]
        e2d = es.rearrange("h (a b) -> h a b", a=2)
        nc.gpsimd.affine_select(
            out=e2d, in_=e2d, compare_op=mybir.AluOpType.not_equal, fill=1.0,
            base=-2 * pc, pattern=[[-1, 2], [0, 64]], channel_multiplier=1,
        )
        e = consts.tile([8, P], F32R, tag=f"e8_{pc}", name=f"e8_{pc}")
        nc.vector.tensor_copy(e, es)
        e8.append(e)
    cstage_ctx.close()   # release staging SBUF before phase A pools
    # ones columns of V (col 64 of each 65-wide head slot)
    v_h = v.rearrange("p j (h e) -> p j h e", e=65)
    nc.gpsimd.memset(v_h[:, :, :, 64:65].bitcast(F32), 1.0)

    # ---------------- Phase A: QT, KT, V projections ----------------
    # Two passes over halves of the contraction dim C to bound SBUF.
    with ExitStack() as actx:
        a_x = actx.enter_context(tc.tile_pool(name="phaseA_x", bufs=5))
        a_w = actx.enter_context(tc.tile_pool(name="phaseA_w", bufs=4))
        a_psum = actx.enter_context(
            tc.tile_pool(name="phaseA_ps", bufs=3, space="PSUM")
        )

        for kp in range(2):
            xth = []
            wh = []
            for cc in range(4):
                g = (kp * 4 + cc) * P
                xc = a_x.tile([P, T], F32R, tag="xt", name=f"x{kp}{cc}")
                for ib in range(4):
                    nc.sync.dma_start(
                        xc[:, ib * 512:(ib + 1) * 512],
                        xt[g:g + P, ib * 512:(ib + 1) * 512],
                    )
                wc = a_w.tile([P, 3 * CG], F32R, tag="w", name=f"w{kp}{cc}")
                for part in range(3):
                    nc.sync.dma_start(
                        wc[:, part * CG:(part + 1) * CG],
                        wqkvt[g:g + P, part * CG:(part + 1) * CG],
                    )
                xth.append(xc)
                wh.append(wc)

            # QT rows m<4 from Wq cols, KT rows m>=4 from Wk cols
            for m in range(8):
                dst, mc = (qt, m) if m < 4 else (kt, m - 4)
                wcol = (0 if m < 4 else CG) + mc * P
                for ib in range(4):
                    ps = a_psum.tile([P, 512], F32, tag="aps")
                    for cc in range(4):
                        nc.tensor.matmul(
                            ps, wh[cc][:, wcol:wcol + P],
                            xth[cc][:, ib * 512:(ib + 1) * 512],
                            start=(cc == 0), stop=(cc == 3),
                        )
                    dslice = dst[:, mc, ib * 512:(ib + 1) * 512]
                    if kp == 0:
                        nc.scalar.copy(dslice, ps)
                    else:
                        nc.vector.tensor_add(out=dslice, in0=ps, in1=dslice)
            # V tiles
            for it in range(NJT):
                ps = a_psum.tile([P, 512], F32, tag="aps")
                for cc in range(4):
                    nc.tensor.matmul(
                        ps, xth[cc][:, it * P:(it + 1) * P],
                        wh[cc][:, 2 * CG:3 * CG],
                        start=(cc == 0), stop=(cc == 3),
                    )
                vdst = v_h[:, it, :, 0:64]
                psv = ps.rearrange("p (h e) -> p h e", e=64)
                if kp == 0:
                    nc.scalar.copy(vdst, psv)
                else:
                    nc.vector.tensor_add(out=vdst, in0=psv, in1=vdst)

    # w_proj slice (loaded once; used in phase B) — own pool, after phase A
    # pools released their SBUF
    wpt_pool = ctx.enter_context(tc.tile_pool(name="wpt", bufs=1))
    wpt_sb = wpt_pool.tile([P, 4, C], F32R, tag="wpt")
    for pc in range(4):
        for nb in range(2):
            nc.sync.dma_start(
                wpt_sb[:, pc, nb * 512:(nb + 1) * 512],
                wpt[pc * P:(pc + 1) * P, nb * 512:(nb + 1) * 512],
            )

    # ---------------- Phase B: attention ----------------
    st_ps = ctx.enter_context(tc.tile_pool(name="st_ps", bufs=2, space="PSUM"))
    yt_ps_pool = ctx.enter_context(tc.tile_pool(name="yt_ps", bufs=1, space="PSUM"))
    misc_ps = ctx.enter_context(tc.tile_pool(name="misc_ps", bufs=1, space="PSUM"))
    pt_pool = ctx.enter_context(tc.tile_pool(name="pt", bufs=3))
    sc_pool = ctx.enter_context(tc.tile_pool(name="sc", bufs=1))
    d_pool = ctx.enter_context(tc.tile_pool(name="d", bufs=2))
    r_pool = ctx.enter_context(tc.tile_pool(name="r", bufs=1))
    o_pool = ctx.enter_context(tc.tile_pool(name="o", bufs=2))

    for qi in range(NQI):
        njt = 4 * qi + 4          # key tiles in causal range for this block
        d_q = d_pool.tile([8, 512], F32, tag="dq")   # denoms, row = head
        scratch = {}              # per-head unnormalized [Y_h; denom]
        for hp in range(4):       # head pairs -> partition rows 0-63 / 64-127
            yt_tiles = [
                yt_ps_pool.tile([65, 512], F32, tag=f"yt{s}", name=f"yt{s}")
                for s in range(2)
            ]
            for j in range(njt):
                for s in range(2):
                    h = 2 * hp + s
                    r0 = s * 64
                    diag = j >= 4 * qi
                    st = st_ps.tile([P, 512], F32, tag=f"st{s}")
                    nc.tensor.matmul(
                        st, kt[r0:r0 + 64, hp, j * P:(j + 1) * P],
                        qt[r0:r0 + 64, hp, qi * 512:(qi + 1) * 512],
                        start=True, stop=not diag,
                    )
                    if diag:
                        nc.tensor.matmul(
                            st, neg_i, notvalid[j - 4 * qi],
                            start=False, stop=True,
                        )
                    pt = pt_pool.tile([P, 512], F32R, tag=f"pt{s}")
                    nc.scalar.activation(
                        pt, st, mybir.ActivationFunctionType.Exp, scale=0.125
                    )
                    nc.tensor.matmul(
                        yt_tiles[s], v[:, j, h * 65:(h + 1) * 65], pt,
                        start=(j == 0), stop=(j == njt - 1),
                    )
            for s in range(2):
                h = 2 * hp + s
                # unnormalized [Y_h; denom] -> SBUF scratch, then DMA the
                # denom row into d_q (DMA writes any partition; engines
                # can only address 32-aligned partition bases)
                sc = sc_pool.tile([65, 512], F32R, tag=f"sc{h}", name=f"sc{h}")
                nc.scalar.copy(sc, yt_tiles[s][:, :])
                nc.sync.dma_start(d_q[h:h + 1, :], sc[64:65, :].bitcast(F32))
                scratch[h] = sc

        # normalize this query block: R = e8^T @ (1/denoms)
        r_q = r_pool.tile([8, 512], F32, tag="rq")
        nc.vector.reciprocal(r_q, d_q)
        r_qr = r_pool.tile([8, 512], F32R, tag="rqr")
        nc.vector.tensor_copy(r_qr, r_q)
        for pc in range(4):
            rps = misc_ps.tile([P, 512], F32, tag="rps")
            nc.tensor.matmul(rps, e8[pc], r_qr, start=True, stop=True)
            for s in range(2):
                h = 2 * pc + s
                nc.vector.tensor_mul(
                    out=ytu[s * 64:s * 64 + 64, pc, qi * 512:(qi + 1) * 512],
                    in0=scratch[h][0:64, :],
                    in1=rps[s * 64:s * 64 + 64, :],
                )

        # projection for this query block's 4 row tiles
        for it in range(4 * qi, 4 * qi + 4):
            for nb in range(2):
                ops = misc_ps.tile([P, 512], F32, tag="proj")
                for pc in range(4):
                    nc.tensor.matmul(
                        ops, ytu[:, pc, it * P:(it + 1) * P],
                        wpt_sb[:, pc, nb * 512:(nb + 1) * 512],
                        start=(pc == 0), stop=(pc == 3),
                    )
                osb = o_pool.tile([P, 512], F32, tag="osb")
                nc.scalar.copy(osb, ops)
                nc.sync.dma_start(
                    out[it * P:(it + 1) * P, nb * 512:(nb + 1) * 512], osb
                )


def _prep_inputs(x, w_qkv, w_proj):
    """Build the 8 per-core input maps (host-side sharding + transposes)."""
    xts = [np.ascontiguousarray(x[b].T) for b in range(B)]
    wqkvts, wpts = [], []
    for hg in range(HG):
        s = hg * CG
        wq = w_qkv[s:s + CG]
        wk = w_qkv[C + s:C + s + CG]
        wv = w_qkv[2 * C + s:2 * C + s + CG]
        wqkvts.append(np.ascontiguousarray(np.concatenate([wq, wk, wv], 0).T))
        wpts.append(np.ascontiguousarray(w_proj[:, s:s + CG].T))
    in_maps = []
    for c in range(8):
        b, hg = c // 2, c % 2
        in_maps.append({"xt": xts[b], "wqkvt": wqkvts[hg], "wpt": wpts[hg]})
    return in_maps


def kernel(x, w_qkv, w_proj):
    x = np.asarray(x, dtype=np.float32)
    w_qkv = np.asarray(w_qkv, dtype=np.float32)
    w_proj = np.asarray(w_proj, dtype=np.float32)

    if "nc" not in _CACHE:
        _CACHE["nc"] = _build_core_program()
    nc = _CACHE["nc"]

    in_maps = _prep_inputs(x, w_qkv, w_proj)
    res = run_bass_kernel_spmd(nc, in_maps, core_ids=list(range(8)))
    outs = [r["out"] for r in res.results]
    full = np.empty((B, T, C), dtype=np.float32)
    for b in range(B):
        full[b] = outs[2 * b] + outs[2 * b + 1]
    return full



# revision 6
# speedup vs baseline: 2.0982x; 2.0982x over previous
"""Causal self-attention on 8 TRN2 NeuronCores (bf16 PE pipeline).

Problem: x[4, 2048, 1024], w_qkv[3072, 1024], w_proj[1024, 1024],
16 heads x 64 dims, causal softmax attention, output [4, 2048, 1024].

Sharding: core c handles (batch b = c//2, head-group hg = c%2).
Each head-group = 8 heads = 512 channels. Tensor-parallel over heads:
each core computes a *partial* projection output [2048, 1024]; the host
sums the two head-group partials per batch (the "all-reduce" of TP).

Per-core dataflow (all matmuls bf16 -> fp32 PSUM):
  Phase A:  QT = Wq @ X^T   [512, 2048]   (head dims on partitions)
            KT = Wk @ X^T   [512, 2048]
            V  = X @ Wv^T   [2048, 512]   (+ ones column per head)
            single pass over the full contraction dim (8 PSUM-accumulated
            matmuls per output tile), PSUM->SBUF bf16 copies on ACT.
  Phase B (per 512-query block QI, head pair hp, key tile j):
            ST pair = K_h^T Q_h for h=2hp,2hp+1 -> one [128,1024] PSUM
            (2 banks); diagonal tiles compute only the causally valid
            query range (width 512-128*o).
            PT = exp(0.125 * ST) in ONE ACT op over both heads -> bf16;
            diagonal tiles then zero the q<k staircase via gpsimd
            affine_select (c >= p keeps).
            YT_h += [V_h | 1]^T @ PT_h  (accumulate over key tiles;
            row 64 = softmax denominators).
  Normalize: per qi: denoms -> 1/d (DVE), R = e8^T @ r broadcasts over
            the 64 partition rows of each head; ytu = scratch * R (bf16).
  Proj:     out = YT^T-contracted with w_proj slice -> [2048, 1024]
            partial, PSUM->SBUF copy on gpsimd, DMA out.
"""

import numpy as np
from contextlib import ExitStack

import concourse.bass as bass
import concourse.tile as tile
from concourse import bacc, mybir
from concourse.bass_utils import run_bass_kernel_spmd

B, T, C, H, D = 4, 2048, 1024, 16, 64
HG = 2                 # head groups (tensor-parallel ways)
HPG = H // HG          # 8 heads per group
CG = HPG * D           # 512 channels per group
P = 128
NQI = T // 512         # 4 query blocks
NJT = T // P           # 16 key tiles
F32 = mybir.dt.float32
F32R = mybir.dt.float32r
BF16 = mybir.dt.bfloat16

_CACHE = {}


def _build_core_program():
    nc = bacc.Bacc("TRN2", target_bir_lowering=False, debug=False, num_devices=8)
    xt = nc.dram_tensor("xt", [C, T], BF16, kind="ExternalInput").ap()
    wqkvt = nc.dram_tensor("wqkvt", [C, 3 * CG], BF16, kind="ExternalInput").ap()
    wpt = nc.dram_tensor("wpt", [CG, C], BF16, kind="ExternalInput").ap()
    out = nc.dram_tensor("out", [T, C], F32, kind="ExternalOutput").ap()

    with tile.TileContext(nc) as tc:
        with ExitStack() as ctx:
            _attention(ctx, tc, xt, wqkvt, wpt, out)
    nc.compile()
    return nc


def _attention(ctx, tc, xt, wqkvt, wpt, out):
    nc = tc.nc

    persist = ctx.enter_context(tc.tile_pool(name="persist", bufs=1))
    qt = persist.tile([P, 4, T], BF16, tag="qt")       # QT[c*128+p, i] at [p, c, i]
    kt = persist.tile([P, 4, T], BF16, tag="kt")
    v = persist.tile([P, NJT, HPG * 65], BF16, tag="v")  # [V_h | 1] per key tile
    ytu = persist.tile([P, 4, T], BF16, tag="ytu")     # normalized YT

    consts = ctx.enter_context(tc.tile_pool(name="consts", bufs=1))
    cstage_ctx = ExitStack()
    stage = cstage_ctx.enter_context(tc.tile_pool(name="cstage", bufs=1))
    # E matrices: e8[pc][h, c] = 1 iff chunk-pc channel c belongs to head h
    e8 = []
    for pc in range(4):
        es = stage.tile([8, P], F32, tag="cste", name=f"e8s{pc}")
        nc.gpsimd.memset(es, 0.0)
        # row h, cols: head index of col c is 2*pc + c//64 -> fill 1 on match:
        # (h - 2*pc - c//64) == 0.  pattern [[-1,2],[0,64]] over free [2, 64]
        e2d = es.rearrange("h (a b) -> h a b", a=2)
        nc.gpsimd.affine_select(
            out=e2d, in_=e2d, compare_op=mybir.AluOpType.not_equal, fill=1.0,
            base=-2 * pc, pattern=[[-1, 2], [0, 64]], channel_multiplier=1,
        )
        e = consts.tile([8, P], F32R, tag=f"e8_{pc}", name=f"e8_{pc}")
        nc.vector.tensor_copy(e, es)
        e8.append(e)
    cstage_ctx.close()
    # ones columns of V (col 64 of each 65-wide head slot); bf16 1.0 = 0x3f80
    v_h = v.rearrange("p j (h e) -> p j h e", e=65)
    nc.gpsimd.memset(v_h[:, :, :, 64:65].bitcast(mybir.dt.uint16), 0x3F80)

    # ---------------- Phase A: QT, KT, V projections ----------------
    with ExitStack() as actx:
        a_x = actx.enter_context(tc.tile_pool(name="phaseA_x", bufs=1))
        a_w = actx.enter_context(tc.tile_pool(name="phaseA_w", bufs=1))
        a_psum = actx.enter_context(
            tc.tile_pool(name="phaseA_ps", bufs=4, space="PSUM")
        )
        xsb = a_x.tile([P, 8, T], BF16, tag="xsb")
        wsb = a_w.tile([P, 8, 3 * CG], BF16, tag="wsb")
        for g in range(8):
            nc.sync.dma_start(xsb[:, g, :], xt[g * P:(g + 1) * P, :])
            nc.sync.dma_start(wsb[:, g, :], wqkvt[g * P:(g + 1) * P, :])

        # QT rows m<4 from Wq cols, KT rows m>=4 from Wk cols
        for m in range(8):
            dst, mc = (qt, m) if m < 4 else (kt, m - 4)
            wcol = (0 if m < 4 else CG) + mc * P
            for ib in range(4):
                ps = a_psum.tile([P, 512], F32, tag="aps")
                for g in range(8):
                    nc.tensor.matmul(
                        ps, wsb[:, g, wcol:wcol + P],
                        xsb[:, g, ib * 512:(ib + 1) * 512],
                        start=(g == 0), stop=(g == 7),
                    )
                nc.scalar.copy(dst[:, mc, ib * 512:(ib + 1) * 512], ps)
        # V tiles
        for it in range(NJT):
            ps = a_psum.tile([P, 512], F32, tag="aps")
            for g in range(8):
                nc.tensor.matmul(
                    ps, xsb[:, g, it * P:(it + 1) * P],
                    wsb[:, g, 2 * CG:3 * CG],
                    start=(g == 0), stop=(g == 7),
                )
            psv = ps.rearrange("p (h e) -> p h e", e=64)
            nc.scalar.copy(v_h[:, it, :, 0:64], psv)

    # w_proj slice (loaded once; used in phase B)
    wpt_pool = ctx.enter_context(tc.tile_pool(name="wpt", bufs=1))
    wpt_sb = wpt_pool.tile([P, 4, C], BF16, tag="wpt")
    for pc in range(4):
        nc.sync.dma_start(wpt_sb[:, pc, :], wpt[pc * P:(pc + 1) * P, :])

    # ---------------- Phase B: attention ----------------
    st_ps = ctx.enter_context(tc.tile_pool(name="st_ps", bufs=2, space="PSUM"))
    yt_ps_pool = ctx.enter_context(tc.tile_pool(name="yt_ps", bufs=1, space="PSUM"))
    proj_ps = ctx.enter_context(tc.tile_pool(name="proj_ps", bufs=2, space="PSUM"))
    pt_pool = ctx.enter_context(tc.tile_pool(name="pt", bufs=3))
    sc_pool = ctx.enter_context(tc.tile_pool(name="sc", bufs=1))
    d_pool = ctx.enter_context(tc.tile_pool(name="d", bufs=2))
    r_pool = ctx.enter_context(tc.tile_pool(name="r", bufs=1))
    o_pool = ctx.enter_context(tc.tile_pool(name="o", bufs=2))

    for qi in range(NQI):
        njt = 4 * qi + 4          # key tiles in causal range for this block
        d_q = d_pool.tile([8, 512], F32, tag="dq")   # denoms, row = head
        scratch = {}              # per-head unnormalized [Y_h; denom]
        for hp in range(4):       # head pairs -> partition rows 0-63 / 64-127
            yt_tiles = [
                yt_ps_pool.tile([65, 512], F32, tag=f"yt{s}", name=f"yt{s}")
                for s in range(2)
            ]
            for j in range(njt):
                o = j - 4 * qi     # diagonal offset (>=0 on causal diagonal)
                off = 128 * o if o > 0 else 0   # first valid query column
                W = 512 - off
                st = st_ps.tile([P, 1024], F32, tag="st")
                for s in range(2):
                    r0 = s * 64
                    nc.tensor.matmul(
                        st[:, s * 512 + off:(s + 1) * 512],
                        kt[r0:r0 + 64, hp, j * P:(j + 1) * P],
                        qt[r0:r0 + 64, hp, qi * 512 + off:(qi + 1) * 512],
                        start=True, stop=True,
                    )
                pt = pt_pool.tile([P, 1024], BF16, tag="pt")
                st3 = st.rearrange("p (s q) -> p s q", s=2)[:, :, off:]
                pt3 = pt.rearrange("p (s q) -> p s q", s=2)[:, :, off:]
                nc.scalar.activation(
                    pt3, st3, mybir.ActivationFunctionType.Exp, scale=0.125
                )
                if o >= 0:
                    # zero the still-invalid staircase: keep where q-col >= p
                    nc.gpsimd.affine_select(
                        out=pt3, in_=pt3, compare_op=mybir.AluOpType.is_ge,
                        fill=0.0, base=0, pattern=[[0, 2], [1, W]],
                        channel_multiplier=-1,
                    )
                for s in range(2):
                    h = 2 * hp + s
                    nc.tensor.matmul(
                        yt_tiles[s][:, off:512],
                        v[:, j, h * 65:(h + 1) * 65],
                        pt[:, s * 512 + off:(s + 1) * 512],
                        start=(j == 0), stop=(j == njt - 1),
                    )
            for s in range(2):
                h = 2 * hp + s
                # unnormalized [Y_h; denom] -> SBUF scratch, then DMA the
                # denom row into d_q (DMA writes any partition; engines
                # can only address 32-aligned partition bases)
                sc = sc_pool.tile([65, 512], F32R, tag=f"sc{h}", name=f"sc{h}")
                nc.vector.tensor_copy(sc, yt_tiles[s][:, :])
                nc.sync.dma_start(d_q[h:h + 1, :], sc[64:65, :].bitcast(F32))
                scratch[h] = sc

        # normalize this query block: R = e8^T @ (1/denoms)
        r_q = r_pool.tile([8, 512], F32, tag="rq")
        nc.vector.reciprocal(r_q, d_q)
        r_qr = r_pool.tile([8, 512], F32R, tag="rqr")
        nc.vector.tensor_copy(r_qr, r_q)
        for pc in range(4):
            rps = proj_ps.tile([P, 512], F32, tag="pj", name="rps")
            nc.tensor.matmul(rps, e8[pc], r_qr, start=True, stop=True)
            for s in range(2):
                h = 2 * pc + s
                nc.vector.tensor_mul(
                    out=ytu[s * 64:s * 64 + 64, pc, qi * 512:(qi + 1) * 512],
                    in0=scratch[h][0:64, :],
                    in1=rps[s * 64:s * 64 + 64, :],
                )

        # projection for this query block's 4 row tiles
        for it in range(4 * qi, 4 * qi + 4):
            for nb in range(2):
                ops = proj_ps.tile([P, 512], F32, tag="pj", name="ops")
                for pc in range(4):
                    nc.tensor.matmul(
                        ops, ytu[:, pc, it * P:(it + 1) * P],
                        wpt_sb[:, pc, nb * 512:(nb + 1) * 512],
                        start=(pc == 0), stop=(pc == 3),
                    )
                osb = o_pool.tile([P, 512], F32, tag="osb")
                nc.vector.tensor_copy(osb, ops)
                nc.sync.dma_start(
                    out[it * P:(it + 1) * P, nb * 512:(nb + 1) * 512], osb
                )


def _prep_inputs(x, w_qkv, w_proj):
    """Build the 8 per-core input maps (host-side sharding + transposes)."""
    import ml_dtypes
    bf16 = ml_dtypes.bfloat16
    xts = [np.ascontiguousarray(x[b].T).astype(bf16) for b in range(B)]
    wqkvts, wpts = [], []
    for hg in range(HG):
        s = hg * CG
        wq = w_qkv[s:s + CG]
        wk = w_qkv[C + s:C + s + CG]
        wv = w_qkv[2 * C + s:2 * C + s + CG]
        wqkvts.append(
            np.ascontiguousarray(np.concatenate([wq, wk, wv], 0).T).astype(bf16)
        )
        wpts.append(np.ascontiguousarray(w_proj[:, s:s + CG].T).astype(bf16))
    in_maps = []
    for c in range(8):
        b, hg = c // 2, c % 2
        in_maps.append({"xt": xts[b], "wqkvt": wqkvts[hg], "wpt": wpts[hg]})
    return in_maps


def kernel(x, w_qkv, w_proj):
    x = np.asarray(x, dtype=np.float32)
    w_qkv = np.asarray(w_qkv, dtype=np.float32)
    w_proj = np.asarray(w_proj, dtype=np.float32)

    if "nc" not in _CACHE:
        _CACHE["nc"] = _build_core_program()
    nc = _CACHE["nc"]

    in_maps = _prep_inputs(x, w_qkv, w_proj)
    res = run_bass_kernel_spmd(nc, in_maps, core_ids=list(range(8)))
    outs = [r["out"] for r in res.results]
    full = np.empty((B, T, C), dtype=np.float32)
    for b in range(B):
        full[b] = outs[2 * b] + outs[2 * b + 1]
    return full
